# revision 1
# baseline (speedup 1.0000x reference)
"""CenterLoss Trainium2 kernel.

loss = (sum_i clamp(||x_i - centers[labels_i]||^2, 1e-12, 1e12)
        + BS*(C_OUT-1)*1e-12) / BS

The reference materializes the full [BS, C_OUT] distance matrix, masks it
with one-hot(labels), clamps and sums.  Masking keeps exactly one distance
per row (the label column); the other BS*(C_OUT-1) masked-out zeros each
clamp to 1e-12, a deterministic constant added on the host.

Strategy: data-parallel over batch across 8 NeuronCores, centers
replicated.  Each core indirect-DMA-gathers its 1024 label rows from the
centers table in DRAM, computes per-row squared distances with DVE, clamps,
and writes 1024 partial row sums.  Host sums the partials in float64.
"""

import numpy as np

BS, C_OUT, D = 8192, 50000, 64
N_CORES = 8
ROWS = BS // N_CORES  # rows per core
P = 128  # SBUF partitions
RPP = ROWS // P  # rows per partition
CLAMP_MIN, CLAMP_MAX = 1e-12, 1e12

_CACHE = {}


def _build_program():
    import concourse.bacc as bacc
    import concourse.bass as bass
    import concourse.mybir as mybir
    import concourse.tile as tile

    nc = bacc.Bacc(
        "TRN2", target_bir_lowering=False, debug=False, num_devices=N_CORES
    )

    x_d = nc.dram_tensor("x", [ROWS, D], mybir.dt.float32, kind="ExternalInput")
    lab_d = nc.dram_tensor("labels", [ROWS], mybir.dt.int32, kind="ExternalInput")
    cen_d = nc.dram_tensor(
        "centers", [C_OUT, D], mybir.dt.float32, kind="ExternalInput"
    )
    out_d = nc.dram_tensor("out", [P, RPP], mybir.dt.float32, kind="ExternalOutput")

    f32 = mybir.dt.float32

    with tile.TileContext(nc) as tc:
        with tc.tile_pool(name="sbuf", bufs=1) as pool:
            x_t = pool.tile([P, RPP * D], f32)
            lab_t = pool.tile([P, RPP], mybir.dt.int32)
            c_t = pool.tile([P, RPP * D], f32)
            d_t = pool.tile([P, RPP * D], f32)
            sq_t = pool.tile([P, RPP * D], f32)
            rs_t = pool.tile([P, RPP], f32)
            cl_t = pool.tile([P, RPP], f32)

            # x rows r = p*RPP + j land at partition p, columns [j*D, (j+1)*D)
            nc.sync.dma_start(
                out=x_t[:], in_=x_d[:].rearrange("(p n) m -> p (n m)", p=P)
            )
            nc.sync.dma_start(
                out=lab_t[:], in_=lab_d[:].rearrange("(p n) -> p n", p=P)
            )
            # gather centers[labels]: index k=p*RPP+j pulls 64 contiguous f32
            # into partition p, columns [j*D, (j+1)*D) -- matches x layout
            nc.gpsimd.indirect_dma_start(
                out=c_t[:],
                out_offset=None,
                in_=cen_d[:],
                in_offset=bass.IndirectOffsetOnAxis(ap=lab_t[:], axis=0),
            )
            nc.vector.tensor_tensor(
                out=d_t[:], in0=x_t[:], in1=c_t[:], op=mybir.AluOpType.subtract
            )
            nc.vector.tensor_tensor(
                out=sq_t[:], in0=d_t[:], in1=d_t[:], op=mybir.AluOpType.mult
            )
            nc.vector.reduce_sum(
                out=rs_t[:],
                in_=sq_t[:].rearrange("p (n m) -> p n m", m=D),
                axis=mybir.AxisListType.X,
            )
            nc.vector.tensor_scalar(
                out=cl_t[:],
                in0=rs_t[:],
                scalar1=CLAMP_MIN,
                scalar2=CLAMP_MAX,
                op0=mybir.AluOpType.max,
                op1=mybir.AluOpType.min,
            )
            nc.sync.dma_start(out=out_d[:], in_=cl_t[:])

    nc.compile()
    return nc


def _get_program():
    if "nc" not in _CACHE:
        _CACHE["nc"] = _build_program()
    return _CACHE["nc"]


def kernel(x, labels, centers, trace=False):
    from concourse.bass_utils import run_bass_kernel_spmd

    nc = _get_program()

    x = np.ascontiguousarray(np.asarray(x, dtype=np.float32))
    labels_i32 = np.ascontiguousarray(np.asarray(labels, dtype=np.int32))
    centers = np.ascontiguousarray(np.asarray(centers, dtype=np.float32))

    in_maps = [
        {
            "x": x[i * ROWS : (i + 1) * ROWS],
            "labels": labels_i32[i * ROWS : (i + 1) * ROWS],
            "centers": centers,
        }
        for i in range(N_CORES)
    ]

    res = run_bass_kernel_spmd(
        nc, in_maps, core_ids=list(range(N_CORES)), trace=trace
    )

    total = np.float64(0.0)
    for r in res.results:
        total += np.sum(r["out"], dtype=np.float64)
    # masked-out entries: BS*(C_OUT-1) zeros, each clamped to 1e-12
    total += np.float64(BS) * np.float64(C_OUT - 1) * 1e-12
    loss = np.float32(total / BS)

    if trace:
        _CACHE["last_exec_time_ns"] = res.exec_time_ns
        _CACHE["last_results"] = res
    return np.array(loss, dtype=np.float32)
